# revision 1
# baseline (speedup 1.0000x reference)
"""Complementary gray-code structured-light decoder on 8 Trainium2 NeuronCores.

kernel(images: [24, 2048, 2448] f32) -> [2048, 2448, 2] f32

Sharding: H rows split across 8 cores (256 rows each), data-parallel. Per core
the 256x2448 slab is processed as 8 tiles of [128 rows x 612 cols].

Math (per pixel, direction d in {col,row}; frames 4d..4d+3 = phase steps,
frames 8+8d..15+8d = gray codes):
  c = i0 - i2 ; s = i1 - i3                  (f32; eps terms dropped)
  thr = (sum of the 8 phase frames)/8        (PE matmul accumulate)
  b_i = gc_i > thr ; x_i = b_0^..^b_i        (gray decode, bf16 planes)
  A  = sign(c) * atan(s/(|c|+1e-35))         (ACT Arctan is full-domain)
  e  = x_7 XOR (c >= 0)
  out = mask * ( -S'*A + sum_i 2^(10-i)*x_i - 8*e + 8 ),  S' = 16/(2pi)
  mask = (s^2+c^2 > T_EFF) in either direction
Derivation: k2 = k1 + x7 collapses the three unwrap regions of the reference
into 8*v2 + 8*[x7 == (c>=0)] with v2 = sum x_i 2^(7-i); the atan2 quadrant
terms fold into sign(c)*atan(s/|c|) with no select needed.

Engine split per tile: PE accumulates the whole linear combine (8 gray powers
+ atan term + e term) as bf16 diagonal matmuls into PSUM, plus the f32
threshold sum; DVE does the 16-plane f32 compare, the 7-step XOR cascade and
4 small ops; ACT does Abs/Ln/Exp (reciprocal), Sign, Arctan, Squares and the
PSUM evacuations; GPSIMD does the c/s subtracts, u = s*(1/|c|), and q.
"""
import numpy as np

import concourse.bass as bass
import concourse.mybir as mybir
import concourse.tile as tile
from concourse.vector_clock import ScopedClock
from concourse.bass_utils import run_bass_kernel_spmd

# ---------------- constants ----------------
H, W = 2048, 2448
NFRAMES = 24
NCORES = 8
ROWS_PER_CORE = H // NCORES          # 256
F = 612                              # tile free width; W = 4*F
FH = F // 2                          # 306, PSUM-bank-sized half
F2 = 2 * F
NT_R = ROWS_PER_CORE // 128          # 2
NT_C = W // F                        # 4

T_EFF = 0.010000010952353477         # (q > T_EFF) == (0.5*sqrt(q) > 0.05f)
S_PRIME = float(np.float32(16.0 / (2.0 * np.float64(np.pi))))

# PE diagonal weights: x_i powers, then -4 (Amp term), then +8 (x7*sgc term)
WB_VALS = [1024.0, 512.0, 256.0, 128.0, 64.0, 32.0, 16.0, 8.0, -4.0, 8.0]
WB_A, WB_E = 8, 9

f32 = mybir.dt.float32
bf16 = mybir.dt.bfloat16
OP = mybir.AluOpType
AF = mybir.ActivationFunctionType

_ctr = [0]


def _sanitize_waits(nc):
    """This walrus build rejects instructions carrying >1 sync wait. Move
    excess waits onto fresh same-engine NOPs inserted just before."""
    for f in nc.m.functions:
        for bb in f.blocks:
            il = bb.instructions
            i = 0
            while i < len(il):
                ins = il[i]
                si = getattr(ins, "sync_info", None)
                waits = list(si.on_wait) if si is not None and si.on_wait else []
                if len(waits) > 1:
                    si.on_wait = [waits[-1]]
                    ins.sync_info = si
                    for w in waits[:-1]:
                        _ctr[0] += 1
                        n = mybir.InstNoOp(name=f"waitsplit_{_ctr[0]}")
                        n.engine = ins.engine
                        n.sync_info = mybir.SyncInfo(on_wait=[w], on_update=[])
                        il.insert(i, n)
                        i += 1
                i += 1


class _SafeTileContext(tile.TileContext):
    """TileContext whose exit drain splits its sem waits across SP NOPs
    (the drain is emitted inside __exit__, before _sanitize_waits can run)."""

    def _drain_and_barrier(self, tick_clock, wait_clock):
        nop_inst = self.nc.sync.nop()
        wait_clock.add_sem_waits(
            nop_inst.ins, ScopedClock({None: tick_clock.global_clock})
        )
        si = nop_inst.ins.sync_info
        waits = list(si.on_wait) if si is not None else []
        if len(waits) > 1:
            si.on_wait = waits[:1]
            nop_inst.ins.sync_info = si
            for w in waits[1:]:
                n2 = self.nc.sync.nop()
                n2.ins.sync_info = mybir.SyncInfo(on_wait=[w], on_update=[])
        self.nc.sync.drain()

        self.nc.all_engine_barrier()
        assert self.sems is not None
        popped = self.nc._tile_sem_poison_stack.pop()
        assert popped is self._sem_poison
        self.nc.clear_and_free_semaphores(list(self.sems.allocated().values()))
        self.nc.all_engine_barrier()


def _build_program(sanitize=True):
    import contextlib

    nc = bass.Bass("TRN2", target_bir_lowering=False, debug=False)
    img = nc.dram_tensor("img", [NFRAMES, ROWS_PER_CORE, W], f32, kind="ExternalInput")
    wtsb = nc.dram_tensor("wtsb", [len(WB_VALS), 128, 128], bf16, kind="ExternalInput")
    out = nc.dram_tensor("out", [ROWS_PER_CORE, W, 2], f32, kind="ExternalOutput")

    with _SafeTileContext(nc) as tc, contextlib.ExitStack() as ctx:
        wpool = ctx.enter_context(tc.tile_pool(name="wpool", bufs=1))
        ps_in = ctx.enter_context(tc.tile_pool(name="ps_in", bufs=1))
        gc_in = ctx.enter_context(tc.tile_pool(name="gc_in", bufs=2))
        sb = ctx.enter_context(tc.tile_pool(name="sb", bufs=1))
        outp = ctx.enter_context(tc.tile_pool(name="outp", bufs=2))
        psum = ctx.enter_context(tc.tile_pool(name="psum", bufs=1, space="PSUM"))

        b35 = wpool.tile([128, 1], f32, tag="b35")
        nc.vector.memset(b35[:, :], 1e-35)
        wtb = wpool.tile([128, len(WB_VALS) * 128], bf16, tag="wtb")
        for wi in range(len(WB_VALS)):
            nc.sync.dma_start(out=wtb[:, wi * 128:(wi + 1) * 128], in_=wtsb[wi, :, :])

        def wbslot(i):
            return wtb[:, i * 128:(i + 1) * 128]

        for rb in range(NT_R):
            r0 = rb * 128
            for cb in range(NT_C):
                c0 = cb * F
                # ---------------- loads ----------------
                Xps = ps_in.tile([128, 8 * F], f32, tag="xps")
                nc.sync.dma_start(
                    out=Xps[:, :].rearrange("p (f x) -> p f x", f=8),
                    in_=img[0:8, r0:r0 + 128, c0:c0 + F].rearrange("f p x -> p f x"),
                )
                Xgc = gc_in.tile([128, 16 * F], f32, tag="xgc")
                nc.sync.dma_start(
                    out=Xgc[:, :].rearrange("p (f x) -> p f x", f=16),
                    in_=img[8:24, r0:r0 + 128, c0:c0 + F].rearrange("f p x -> p f x"),
                )

                # ---------------- c/s on GPSIMD, 8-frame sum split ----------
                # cs layout: [c_col | c_row | s_col | s_row]
                cs = sb.tile([128, 4 * F], f32, tag="cs")
                for (k, a, b) in ((0, 0, 2), (1, 4, 6), (2, 1, 3), (3, 5, 7)):
                    nc.gpsimd.tensor_tensor(
                        cs[:, k * F:(k + 1) * F],
                        Xps[:, a * F:(a + 1) * F],
                        Xps[:, b * F:(b + 1) * F],
                        OP.subtract,
                    )
                c_part = cs[:, 0:F2]
                s_part = cs[:, F2:2 * F2]

                t1 = sb.tile([128, 4 * F], f32, tag="t1")
                nc.gpsimd.tensor_tensor(
                    t1[:, :], Xps[:, 0:4 * F], Xps[:, 4 * F:8 * F], OP.add)
                t2 = sb.tile([128, F2], f32, tag="t2")
                nc.vector.tensor_tensor(
                    t2[:, :], t1[:, 0:F2], t1[:, F2:2 * F2], OP.add)
                s8 = sb.tile([128, F], f32, tag="s8")
                nc.vector.tensor_tensor(
                    s8[:, :], t2[:, 0:F], t2[:, F:F2], OP.add)

                # ---------------- gray compare (DVE, 2 STT ops) ------------
                # (8*gc > S8) == (gc > S8/8) exactly; B layout [bit, dir, x]
                B = sb.tile([128, 16 * F], bf16, tag="B")
                Bv = B[:, :].rearrange("p (b d x) -> p b d x", b=8, d=2)
                Xgv = Xgc[:, :].rearrange("p (d b x) -> p d b x", d=2, b=8)
                s8b = (s8[:, :].rearrange("p (o x) -> p o x", o=1)
                       .broadcast_to([128, 8, F]))
                for dd in range(2):
                    nc.vector.scalar_tensor_tensor(
                        Bv[:, :, dd, :], Xgv[:, dd, :, :], 8.0, s8b,
                        OP.mult, OP.is_gt,
                    )

                def bpair(i):
                    return B[:, i * F2:(i + 1) * F2]

                # ---------------- atan path (ACT heavy) ----------------
                ac = sb.tile([128, F2], f32, tag="ac")
                nc.scalar.activation(ac[:, :], c_part, AF.Abs, bias=0.0, scale=1.0)
                lc = sb.tile([128, F2], f32, tag="lc")
                nc.scalar.activation(lc[:, :], ac[:, :], AF.Ln, bias=b35[:, :], scale=1.0)
                ec = sb.tile([128, F2], f32, tag="ec")
                nc.scalar.activation(ec[:, :], lc[:, :], AF.Exp, bias=0.0, scale=-1.0)
                sgc = sb.tile([128, F2], bf16, tag="sgc")
                nc.scalar.activation(sgc[:, :], c_part, AF.Sign, bias=0.0, scale=1.0)
                u1 = sb.tile([128, F2], f32, tag="u1")
                nc.gpsimd.tensor_tensor(u1[:, :], s_part, ec[:, :], OP.mult)
                A2 = sb.tile([128, F2], bf16, tag="A2")
                nc.scalar.activation(A2[:, :], u1[:, :], AF.Arctan, bias=0.0, scale=1.0)
                # Amp = sgc*(1 + S'/4*A2); PE applies weight -4:
                #   -4*Amp = -S'*sgc*A2 - 4*sgc  (atan term + sign decode term)
                pre = sb.tile([128, F2], bf16, tag="pre")
                nc.vector.tensor_scalar(
                    pre[:, :], A2[:, :], S_PRIME / 4.0, 1.0, OP.mult, OP.add)
                Am = sb.tile([128, F2], bf16, tag="Am")
                nc.vector.tensor_tensor(Am[:, :], pre[:, :], sgc[:, :], OP.mult)

                # ---------------- mask ----------------
                # squares reuse ac/lc buffers (dead after the Ln/Exp chain)
                nc.scalar.activation(ac[:, :], c_part, AF.Square, bias=0.0, scale=1.0)
                nc.scalar.activation(lc[:, :], s_part, AF.Square, bias=0.0, scale=1.0)
                # q reuses t1's buffer (t1 is dead after t2)
                nc.gpsimd.tensor_tensor(t1[:, 0:F2], ac[:, :], lc[:, :], OP.add)
                bit_r = sb.tile([128, F], f32, tag="bit_r")
                nc.vector.tensor_scalar(bit_r[:, :], t1[:, F:F2], T_EFF, None, OP.is_gt)
                maskb = sb.tile([128, F], f32, tag="maskb")
                nc.vector.scalar_tensor_tensor(
                    maskb[:, :], t1[:, 0:F], T_EFF, bit_r[:, :], OP.is_gt, OP.max)

                # ------------- XOR cascade + PE combine, interleaved -------
                tps = [
                    psum.tile([128, FH], f32, tag=f"t_{d}{h}", name=f"t_{d}{h}")
                    for d in range(2) for h in range(2)
                ]

                def dh_slice(t, d, h):
                    return t[:, d * F + h * FH: d * F + (h + 1) * FH]

                xtiles = [
                    sb.tile([128, F2], bf16, tag=f"x_{j}", name=f"x_{j}")
                    for j in range(4)
                ]
                xcur = bpair(0)
                for d in range(2):
                    for h in range(2):
                        nc.tensor.matmul(
                            tps[d * 2 + h][:, :], wbslot(0), dh_slice(B, d, h),
                            start=True, stop=False)
                for i in range(1, 8):
                    nxt = xtiles[(i - 1) % 4]
                    nc.vector.tensor_tensor(nxt[:, :], xcur, bpair(i), OP.logical_xor)
                    xcur = nxt[:, :]
                    for d in range(2):
                        for h in range(2):
                            nc.tensor.matmul(
                                tps[d * 2 + h][:, :], wbslot(i), dh_slice(nxt, d, h),
                                start=False, stop=False)
                x7 = xcur  # AP of last cascade tile

                x7s = sb.tile([128, F2], bf16, tag="x7s")
                nc.vector.tensor_tensor(x7s[:, :], x7, sgc[:, :], OP.mult)

                for d in range(2):
                    for h in range(2):
                        nc.tensor.matmul(
                            tps[d * 2 + h][:, :], wbslot(WB_A), dh_slice(Am, d, h),
                            start=False, stop=False)
                        nc.tensor.matmul(
                            tps[d * 2 + h][:, :], wbslot(WB_E), dh_slice(x7s, d, h),
                            start=False, stop=True)

                # ---------------- evac + mask multiply + store -------------
                Tt = sb.tile([128, F2], f32, tag="Tt")
                for d in range(2):
                    for h in range(2):
                        nc.scalar.activation(
                            dh_slice(Tt, d, h), tps[d * 2 + h][:, :],
                            AF.Copy, bias=0.0, scale=1.0)

                o_t = outp.tile([128, F2], f32, tag="o_t")
                ov = o_t[:, :].rearrange("p (x two) -> p two x", two=2)
                nc.vector.scalar_tensor_tensor(
                    ov[:, :, :], Tt[:, :].rearrange("p (d x) -> p d x", d=2),
                    4.0,
                    maskb[:, :].rearrange("p (o x) -> p o x", o=1)
                    .broadcast_to([128, 2, F]),
                    OP.add, OP.mult)
                nc.sync.dma_start(
                    out=out[r0:r0 + 128, c0:c0 + F, :].rearrange("p x two -> p (x two)"),
                    in_=o_t[:, :],
                )

    if sanitize:
        _sanitize_waits(nc)
    return nc


def _weights_b():
    import ml_dtypes
    I = np.eye(128, dtype=np.float32)
    return np.stack([np.float32(v) * I for v in WB_VALS]).astype(ml_dtypes.bfloat16)


_CACHE = {}


def _in_maps(images):
    wtsb = _weights_b()
    maps = []
    for core in range(NCORES):
        r0 = core * ROWS_PER_CORE
        maps.append({
            "img": np.ascontiguousarray(images[:, r0:r0 + ROWS_PER_CORE, :]),
            "wtsb": wtsb,
        })
    return maps


def kernel(images: np.ndarray) -> np.ndarray:
    images = np.ascontiguousarray(np.asarray(images, dtype=np.float32))
    assert images.shape == (NFRAMES, H, W), images.shape
    if "nc" not in _CACHE:
        _CACHE["nc"] = _build_program()
    res = run_bass_kernel_spmd(_CACHE["nc"], _in_maps(images), core_ids=list(range(NCORES)))
    out = np.empty((H, W, 2), dtype=np.float32)
    for core in range(NCORES):
        r0 = core * ROWS_PER_CORE
        out[r0:r0 + ROWS_PER_CORE] = res.results[core]["out"]
    return out


def timed_run(images: np.ndarray):
    """Run once with NTFF tracing; returns max per-core exec_time_ns or None."""
    images = np.ascontiguousarray(np.asarray(images, dtype=np.float32))
    if "nc" not in _CACHE:
        _CACHE["nc"] = _build_program()
    try:
        res = run_bass_kernel_spmd(
            _CACHE["nc"], _in_maps(images), core_ids=list(range(NCORES)),
            trace=True, trace_cores=[0],
        )
        return res.exec_time_ns
    except Exception as exc:
        print(f"timed_run: trace failed ({exc})")
        return None


if __name__ == "__main__":
    rng = np.random.default_rng(0)
    imgs = rng.random((NFRAMES, H, W), dtype=np.float32)
    o = kernel(imgs)
    print("ran:", o.shape, o.dtype, float(np.abs(o).max()))

